# revision 9
# baseline (speedup 1.0000x reference)
"""Joint-entropy (KDE logsumexp over 3x3 windows) Trainium2 kernel, v4.

Math: for each 3x3 window of pixel vectors v_n (C=3 channels),
  out[i,j] = log_norm - (1/9) * sum_n log(S_n),  S_n = sum_m exp(-2*||v_n-v_m||^2)
with log_norm = log(9) + 3*log(sqrt(2*pi)*0.5)  (h = 0.5, logits = -2*d2).

Sharding: 8 cores = 4 batches x 2 row-halves. Each core gets a host-padded
bf16 slab [130, 2, 3, 260] (row-major; plane 0 = x, plane 1 = x shifted one
column left) and produces a [128, 254] fp32 output slab (row 127 garbage,
dropped by the host). All window math is local; no collectives.

Pipeline (absolute-row E planes, 14 plane-slots):
    E0A[p,t,u] = E((p,u),(p,u+t+1))      t in {0,1}   rows 0..127
    E0B[p,t,u] = E((p+1,u),(p+1,u+t+1))  t in {0,1}   rows 1..128
    E1 [p,t,u] = E((p,u),(p+1,u+t-2))    t in 0..4    rows 0..127
    E2 [p,t,u] = E((p,u),(p+2,u+t-2))    t in 0..4    rows 0..126
- Stage B fans across all four compute engines: channel-wide 4D-AP subs and
  two squares-as-muls on VectorE, one square per group on ScalarE, the
  first d2 accumulation on GpSimd, the second on VectorE, Exp on ScalarE.
  E1/E2 are processed in plane-halves so downstream stages start earlier.
- Stage C: per-role window sums as 72 accumulating TensorE matmuls with
  0/1 shift-band stationaries (partition-shifted reads are free); 9 role
  maps in PSUM fp32, 2 roles per bank, one accumulation group per bank.
  A burst of junk matmuls right after the weights land warms the PE HAM
  clock gate so the real stream runs at 2.4 GHz.
- Stage D: Ln(1 + S) per role straight out of PSUM (self term rides the
  ACT affine), bf16 add tree split vector/gpsimd, one tensor_scalar.
- Square/Exp/Ln forced into one ACT table set; all DMAs are 128-partition
  patterns (HWDGE only fans out across the 16 DMA engines for those) with
  3120B-contiguous rows, split across the SP and ACT HWDGE queues.
"""

import dataclasses

import ml_dtypes
import numpy as np

import concourse.bacc as bacc
import concourse.tile as tile
from concourse import mybir
from concourse.bass_utils import run_bass_kernel_spmd

F32 = mybir.dt.float32
BF16 = mybir.dt.bfloat16
AOP = mybir.AluOpType
AF = mybir.ActivationFunctionType

B = 4
C = 3
W = 256
PAD = 2
WT = W + 2 * PAD
ROWS_IN = 130  # 129 real rows + 1 pad row so every X tile is 128 partitions
ROWS_OUT = 127
WOUT = 254
LOG_NORM = float(np.log(9.0) + 3.0 * np.log(np.sqrt(2.0 * np.pi) * 0.5))
N_WARM_MM = 20

# role r = nr*3 + ncol -> (psum bank, slot). Roles 3,4 (nr=1) share a bank
# whose accumulation finishes with the E1 matmul block, so their Lns
# overlap E2 compute; the other banks finish staggered in the E2 block.
ROLE_SLOT = {
    3: (0, 0), 4: (0, 1),
    5: (1, 0), 0: (1, 1),
    1: (2, 0), 2: (2, 1),
    6: (3, 0), 7: (3, 1),
    8: (4, 0),
}


def _role_terms():
    """Per role (nr, ncol): list of 8 terms (tile_name, s, t, c0).

    Term value for window (i, j) = E<tile>[i + s, t, j + c0]."""
    out = {}
    for nr in range(3):
        for ncol in range(3):
            tl = []
            for mr in range(3):
                for mc in range(3):
                    if (mr, mc) == (nr, ncol):
                        continue
                    if mr == nr:
                        dc = abs(mc - ncol)
                        if nr <= 1:
                            tl.append(("E0A", nr, dc - 1, min(ncol, mc)))
                        else:
                            tl.append(("E0B", 1, dc - 1, min(ncol, mc)))
                    elif mr > nr:
                        a = mr - nr
                        dc = mc - ncol
                        tl.append((f"E{a}", nr if a == 1 else 0, dc + 2, ncol))
                    else:
                        a = nr - mr
                        dc = ncol - mc
                        tl.append((f"E{a}", mr if a == 1 else 0, dc + 2, mc))
            assert len(tl) == 8
            out[(nr, ncol)] = tl
    return out


def _ap(ap2, dims):
    """Rebuild a sliced AP's non-partition dims. `ap2` must be a [P, w]
    slice whose offset marks the base element; `dims` is a list of
    [step_elems, count] applied after the partition dim."""
    return dataclasses.replace(ap2, ap=[list(ap2.ap[0])] + [list(d) for d in dims])


class _one_act_table:
    """Force Square/Exp/Ln into natural_log_exp_and_others so the kernel
    needs a single ACT table load (set order/ids preserved)."""

    WANT = "natural_log_exp_and_others"
    FNS = frozenset({AF.Exp, AF.Ln, AF.Square})

    def __enter__(self):
        self._orig = bacc.get_activation_tables

        def patched(arch, _orig=self._orig):
            tabs = dict(_orig(arch))
            if self.WANT in tabs and self.FNS <= tabs[self.WANT]:
                tabs = {
                    k: (v if k == self.WANT else set(v) - self.FNS)
                    for k, v in tabs.items()
                }
            return tabs

        bacc.get_activation_tables = patched
        return self

    def __exit__(self, *exc):
        bacc.get_activation_tables = self._orig
        return False


def _build_program():
    nc = bacc.Bacc("TRN2")
    # xin[r, 0, c, w] = x padded; xin[r, 1, c, w] = x padded, shifted 1 col left
    xin = nc.dram_tensor("xin", (ROWS_IN, 2, C, WT), BF16, kind="ExternalInput")
    wsh = nc.dram_tensor("wsh", (128, 2, 128), BF16, kind="ExternalInput")
    yout = nc.dram_tensor("yout", (128, WOUT), F32, kind="ExternalOutput")

    terms = _role_terms()

    with tile.TileContext(nc) as tc:
        with (
            tc.tile_pool(name="xp", bufs=1) as xp,
            tc.tile_pool(name="dp", bufs=1) as dp,
            tc.tile_pool(name="ep", bufs=1) as ep,
            tc.tile_pool(name="pp", bufs=1, space="PSUM") as pp,
            tc.tile_pool(name="sp", bufs=1) as sp,
        ):
            # ---- weights + inputs (HWDGE on both SP and ACT queues) ------
            WS = xp.tile([128, 2, 128], BF16, tag="wsh")
            nc.scalar.dma_start(out=WS, in_=wsh[:, :, :])
            XX = {}
            for s, eng in ((0, nc.sync), (1, nc.scalar), (2, nc.sync)):
                XX[s] = xp.tile([128, 2, C, WT], BF16, tag=f"xx{s}", name=f"xx{s}")
                eng.dma_start(out=XX[s], in_=xin[s : s + 128, :, :, :])

            # ---- PE warm-up: junk matmuls so HAM reaches 2.4 GHz ---------
            JP = pp.tile([128, 256], F32, tag="junk")
            for _ in range(N_WARM_MM):
                nc.tensor.matmul(
                    JP[:, :],
                    WS[:, 0, :],
                    _ap(WS[:, 0, 0:1].unsqueeze(1), [[1, 256]]),
                    start=True,
                    stop=True,
                    skip_group_check=True,
                )

            # ---- stage B ------------------------------------------------
            # groups: (name, P, nb, anchor_s, partner_s, halves)
            groups = [
                ("E0A", 128, 2, 0, 0, ((0, 2),)),
                ("E0B", 128, 2, 1, 1, ((0, 2),)),
                ("E1", 128, 5, 0, 1, ((0, 3), (3, 5))),
                ("E2", 127, 5, 0, 2, ((0, 3), (3, 5))),
            ]
            E = {}

            def build_group(name, P, nb, s_a, s_p, halves):
                xa, xb = XX[s_a], XX[s_p]
                # D[p, c, t, w], channel-major
                D = dp.tile([P, C, nb, W], BF16, tag=f"d_{name}", name=f"d_{name}")
                anchor = xa[0:P, 0, 0, PAD : PAD + W]  # base [P, w] slice
                if nb == 2:
                    # t=0: dc=+1 via shifted plane; t=1: dc=+2 via base plane
                    nc.vector.tensor_sub(
                        _ap(D[0:P, 0, 0:1, 0:W], [[nb * W, C], [1, W]]),
                        _ap(anchor, [[WT, C], [1, W]]),
                        _ap(xb[0:P, 1, 0, PAD : PAD + W], [[WT, C], [1, W]]),
                    )
                    nc.vector.tensor_sub(
                        _ap(D[0:P, 0, 1:2, 0:W], [[nb * W, C], [1, W]]),
                        _ap(anchor, [[WT, C], [1, W]]),
                        _ap(xb[0:P, 0, 0, PAD + 2 : PAD + 2 + W], [[WT, C], [1, W]]),
                    )
                else:
                    # even planes t=0,2,4 (dc=-2,0,+2) from base plane
                    nc.vector.tensor_sub(
                        _ap(D[0:P, 0, 0:1, 0:W], [[nb * W, C], [2 * W, 3], [1, W]]),
                        _ap(anchor, [[WT, C], [0, 3], [1, W]]),
                        _ap(xb[0:P, 0, 0, PAD - 2 : PAD - 2 + W],
                            [[WT, C], [2, 3], [1, W]]),
                    )
                    # odd planes t=1,3 (dc=-1,+1) from shifted plane
                    nc.vector.tensor_sub(
                        _ap(D[0:P, 0, 1:2, 0:W], [[nb * W, C], [2 * W, 2], [1, W]]),
                        _ap(anchor, [[WT, C], [0, 2], [1, W]]),
                        _ap(xb[0:P, 1, 0, PAD - 2 : PAD - 2 + W],
                            [[WT, C], [2, 2], [1, W]]),
                    )
                Eg = ep.tile([P, nb, W], BF16, tag=f"e_{name}", name=f"e_{name}")
                for hi, (h0, h1) in enumerate(halves):
                    hn = h1 - h0
                    q0 = dp.tile([P, hn, W], BF16, tag=f"q0_{name}_{hi}")
                    nc.vector.tensor_mul(q0, D[0:P, 0, h0:h1, :], D[0:P, 0, h0:h1, :])
                    q1 = dp.tile([P, hn, W], BF16, tag=f"q1_{name}_{hi}")
                    nc.scalar.square(q1, D[0:P, 1, h0:h1, :])
                    q2 = dp.tile([P, hn, W], BF16, tag=f"q2_{name}_{hi}")
                    nc.vector.tensor_mul(q2, D[0:P, 2, h0:h1, :], D[0:P, 2, h0:h1, :])
                    d2a = dp.tile([P, hn, W], BF16, tag=f"d2a_{name}_{hi}")
                    nc.gpsimd.tensor_add(d2a, q0, q1)
                    d2 = dp.tile([P, hn, W], BF16, tag=f"d2_{name}_{hi}")
                    nc.vector.tensor_add(d2, d2a, q2)
                    nc.scalar.activation(Eg[:, h0:h1, :], d2, AF.Exp, scale=-2.0)
                E[name] = Eg

            for g in groups:
                build_group(*g)

            # ---- stage C: role sums on the TensorEngine ------------------
            S = [
                pp.tile([128, 2, WOUT], F32, tag=f"s{k}", name=f"s{k}")
                for k in range(5)
            ]
            started = set()
            order = []
            for tname, _, _, _, _, _ in groups:
                block = []
                for role, tl in terms.items():
                    for term in tl:
                        if term[0] == tname:
                            block.append((role, term))
                # group by stationary within a block (possible LDW dedup),
                # stable so role order (and the r8-last tail) is preserved
                block.sort(key=lambda rt: rt[1][1])
                order.extend(block)
            last_idx = {}
            for idx, (role, _) in enumerate(order):
                last_idx[ROLE_SLOT[role[0] * 3 + role[1]][0]] = idx
            group_p = {g[0]: g[1] for g in groups}
            for idx, (role, (tname, s, t, c0)) in enumerate(order):
                bank, slot = ROLE_SLOT[role[0] * 3 + role[1]]
                k = group_p[tname]
                nc.tensor.matmul(
                    S[bank][:, slot, :],
                    WS[0:k, s, :],
                    E[tname][0:k, t, c0 : c0 + WOUT],
                    start=(bank not in started),
                    stop=(idx == last_idx[bank]),
                    skip_group_check=True,
                )
                started.add(bank)

            # ---- stage D: ln, sum, affine (full 128 partitions) ----------
            LT = sp.tile([128, 9, WOUT], BF16, tag="lt")
            for r in range(9):
                bank, slot = ROLE_SLOT[r]
                nc.scalar.activation(LT[:, r, :], S[bank][:, slot, :], AF.Ln, bias=1.0)
            for eng, a, b_ in (
                (nc.gpsimd, 2, 3),
                (nc.vector, 0, 1),
                (nc.gpsimd, 6, 7),
                (nc.vector, 4, 5),
                (nc.gpsimd, 4, 6),
                (nc.vector, 0, 2),
                (nc.vector, 0, 4),
                (nc.vector, 0, 8),
            ):
                eng.tensor_add(LT[:, a, :], LT[:, a, :], LT[:, b_, :])
            OUT = sp.tile([128, WOUT], F32, tag="out")
            nc.vector.tensor_scalar(
                out=OUT,
                in0=LT[:, 0, :],
                scalar1=-1.0 / 9.0,
                scalar2=LOG_NORM,
                op0=AOP.mult,
                op1=AOP.add,
            )
            nc.sync.dma_start(out=yout[:, :], in_=OUT)
    if not nc.is_finalized():
        with _one_act_table():
            nc.finalize()
    return nc


_PROGRAM = None


def _get_program():
    global _PROGRAM
    if _PROGRAM is None:
        _PROGRAM = _build_program()
    return _PROGRAM


def _make_shift_weights():
    w = np.zeros((128, 2, 128), dtype=ml_dtypes.bfloat16)
    for s in range(2):
        for m in range(128):
            if m + s < 128:
                w[m + s, s, m] = 1.0
    return w


def _shard_inputs(x):
    x = np.asarray(x, dtype=np.float32)
    # [B, rows(257: 256 + pad row), 2(plain, col-shifted), C, WT]
    xp = np.zeros((B, 257, 2, C, WT), dtype=np.float32)
    xp[:, :256, 0, :, PAD : PAD + W] = x.transpose(0, 2, 1, 3)
    xp[:, :, 1, :, : WT - 1] = xp[:, :, 0, :, 1:]
    xp16 = xp.astype(ml_dtypes.bfloat16)
    wsh = _make_shift_weights()
    in_maps = []
    for core in range(8):
        b, half = divmod(core, 2)
        r0 = half * 127
        in_maps.append(
            {
                "xin": np.ascontiguousarray(xp16[b, r0 : r0 + ROWS_IN]),
                "wsh": wsh,
            }
        )
    return in_maps


def _gather(results):
    out = np.empty((B, 254, 254), dtype=np.float32)
    for core in range(8):
        b, half = divmod(core, 2)
        out[b, half * 127 : half * 127 + 127, :] = results[core]["yout"][:127]
    return out


def kernel(x, **_unused):
    nc = _get_program()
    res = run_bass_kernel_spmd(nc, _shard_inputs(x), core_ids=list(range(8)))
    return _gather(res.results)


def kernel_traced(x):
    """Same as kernel() but returns (output, BassKernelResults) with trace."""
    nc = _get_program()
    res = run_bass_kernel_spmd(
        nc, _shard_inputs(x), core_ids=list(range(8)), trace=True
    )
    return _gather(res.results), res


# revision 19
# speedup vs baseline: 1.3590x; 1.3590x over previous
"""Joint-entropy (KDE logsumexp over 3x3 windows) Trainium2 kernel, v5.

Math: for each 3x3 window of pixel vectors v_n (C=3 channels),
  out[i,j] = log_norm - (1/9) * sum_n log(S_n),  S_n = sum_m exp(-2*||v_n-v_m||^2)
with log_norm = log(9) + 3*log(sqrt(2*pi)*0.5)  (h = 0.5, logits = -2*d2).

Sharding: 8 cores = 4 batches x 2 row-halves. Each core gets a host-padded
bf16 slab [130, 2, 3, 260] (row-major; plane 0 = x, plane 1 = x shifted one
column left) and produces a [128, 254] fp32 output slab (row 127 garbage,
dropped by the host). All window math is local; no collectives.

Pipeline (absolute-row E planes, 14 plane-slots):
    E0A[p,t,u] = E((p,u),(p,u+t+1))      t in {0,1}   rows 0..127
    E0B[p,t,u] = E((p+1,u),(p+1,u+t+1))  t in {0,1}   rows 1..128
    E1 [p,t,u] = E((p,u),(p+1,u+t-2))    t in 0..4    rows 0..127
    E2 [p,t,u] = E((p,u),(p+2,u+t-2))    t in 0..4    rows 0..126
- Stage B: per-channel parity-split 3D subs on VectorE (all operands
  4B-aligned via the host-shifted plane -> DVE 2x mode). d2 assembly is
  spread by latency class: E0A (gates the first matmuls) and E2 (gates the
  kernel tail, processed in plane-halves) stay on Vector/Scalar; E0B and
  parts of E1 go to the otherwise-idle GpSimd.
- Stage C: 72 accumulating TensorE matmuls with 0/1 shift-band
  stationaries; 9 role maps in PSUM fp32, 2 roles per bank, one
  accumulation group per bank (groups are bank-granular).
- Stage D: Ln(1 + S) per role from PSUM (self term rides the ACT affine),
  bf16 add tree on VectorE, one tensor_scalar, 128-partition out DMA.
- Square/Exp/Ln forced into one ACT table set; all DMAs are 128-partition
  patterns split across the SP and ACT HWDGE queues.
"""

import dataclasses

import ml_dtypes
import numpy as np

import concourse.bacc as bacc
import concourse.tile as tile
from concourse import mybir
from concourse.bass_utils import run_bass_kernel_spmd

F32 = mybir.dt.float32
BF16 = mybir.dt.bfloat16
AOP = mybir.AluOpType
AF = mybir.ActivationFunctionType

B = 4
C = 3
W = 256
PAD = 2
WT = W + 2 * PAD
ROWS_IN = 130  # 129 real rows + 1 pad row so every X tile is 128 partitions
ROWS_OUT = 127
WOUT = 254
LOG_NORM = float(np.log(9.0) + 3.0 * np.log(np.sqrt(2.0 * np.pi) * 0.5))

# role r = nr*3 + ncol -> (psum bank, slot). Roles 3,4 (nr=1) share a bank
# whose accumulation finishes with the E1 matmul block, so their Lns
# overlap E2 compute; the other banks finish staggered in the E2 block.
ROLE_SLOT = {
    3: (0, 0), 4: (0, 1),
    1: (1, 0), 2: (1, 1),
    6: (2, 0), 7: (2, 1),
    5: (3, 0), 8: (3, 1),
    0: (4, 0),
}


def _role_terms():
    """Per role (nr, ncol): list of 8 terms (tile_name, s, t, c0).

    Term value for window (i, j) = E<tile>[i + s, t, j + c0]."""
    out = {}
    for nr in range(3):
        for ncol in range(3):
            tl = []
            for mr in range(3):
                for mc in range(3):
                    if (mr, mc) == (nr, ncol):
                        continue
                    if mr == nr:
                        dc = abs(mc - ncol)
                        if nr <= 1:
                            tl.append(("E0A", nr, dc - 1, min(ncol, mc)))
                        else:
                            tl.append(("E0B", 1, dc - 1, min(ncol, mc)))
                    elif mr > nr:
                        a = mr - nr
                        dc = mc - ncol
                        tl.append((f"E{a}", nr if a == 1 else 0, dc + 2, ncol))
                    else:
                        a = nr - mr
                        dc = ncol - mc
                        tl.append((f"E{a}", mr if a == 1 else 0, dc + 2, mc))
            assert len(tl) == 8
            out[(nr, ncol)] = tl
    return out


def _ap(ap2, dims):
    """Rebuild a sliced AP's non-partition dims: `ap2` is a [P, w] slice
    whose offset marks the base element; `dims` = [[step_elems, count], ...]
    applied after the partition dim."""
    return dataclasses.replace(ap2, ap=[list(ap2.ap[0])] + [list(d) for d in dims])


class _one_act_table:
    """Force Square/Exp/Ln into natural_log_exp_and_others so the kernel
    needs a single ACT table load (set order/ids preserved)."""

    WANT = "natural_log_exp_and_others"
    FNS = frozenset({AF.Exp, AF.Ln, AF.Square})

    def __enter__(self):
        self._orig = bacc.get_activation_tables

        def patched(arch, _orig=self._orig):
            tabs = dict(_orig(arch))
            if self.WANT in tabs and self.FNS <= tabs[self.WANT]:
                tabs = {
                    k: (v if k == self.WANT else set(v) - self.FNS)
                    for k, v in tabs.items()
                }
            return tabs

        bacc.get_activation_tables = patched
        return self

    def __exit__(self, *exc):
        bacc.get_activation_tables = self._orig
        return False


def _build_program():
    nc = bacc.Bacc("TRN2")
    # xin[r, 0, c, w] = x padded; xin[r, 1, c, w] = same, shifted 1 col left
    xin = nc.dram_tensor("xin", (ROWS_IN, 2, C, WT), BF16, kind="ExternalInput")
    FP8 = mybir.dt.float8e4
    wsh = nc.dram_tensor("wsh", (128, 2, 128), FP8, kind="ExternalInput")
    yout = nc.dram_tensor("yout", (128, WOUT), BF16, kind="ExternalOutput")

    terms = _role_terms()

    with tile.TileContext(nc) as tc:
        with (
            tc.tile_pool(name="xp", bufs=1) as xp,
            tc.tile_pool(name="dp", bufs=1) as dp,
            tc.tile_pool(name="ep", bufs=1) as ep,
            tc.tile_pool(name="pp", bufs=1, space="PSUM") as pp,
            tc.tile_pool(name="sp", bufs=1) as sp,
        ):
            # ---- weights + inputs (HWDGE on both SP and ACT queues) ------
            WS = xp.tile([128, 2, 128], FP8, tag="wsh")
            nc.scalar.dma_start(out=WS, in_=wsh[:, :, :])
            XX = {}
            for s, eng in ((0, nc.sync), (1, nc.scalar), (2, nc.sync)):
                XX[s] = xp.tile([128, 2, C, WT], BF16, tag=f"xx{s}", name=f"xx{s}")
                eng.dma_start(out=XX[s], in_=xin[s : s + 128, :, :, :])

            # ---- PE warm-up: junk matmuls into bank 4 (re-zeroed later by
            # its real accumulation group) so HAM reaches 2.4 GHz before the
            # real stream starts ------------------------------------------
            # (emitted right after the weight DMA; they only need WS)
            # ---- stage B + C, interleaved on the PE ----------------------
            # All d2 accumulation happens on the TensorEngine: for each
            # <=512-element chunk of a unit, 3 accumulating identity-matmuls
            # sum the squared channels in PSUM; Exp reads PSUM directly.
            # Squares are split vector/scalar; subs stay on vector.
            # PE emission interleaves each unit's d2-matmuls + its role-sum
            # block so nothing queues behind later-ready work in the PE FIFO.
            E = {}
            S = [
                pp.tile([128, 2, WOUT], F32, tag=f"s{k}", name=f"s{k}")
                for k in range(5)
            ]
            for _ in range(20):
                nc.tensor.matmul(
                    S[4][:, 0, :],
                    WS[:, 0, :],
                    _ap(WS[:, 0, 0:1], [[1, WOUT]]),
                    start=True,
                    stop=True,
                    skip_group_check=True,
                )
            # Build role-sum matmul descriptors. Terms of the two roles
            # sharing a PSUM bank that use the same stationary (shift s) and
            # the same E tile fuse into ONE N=508 matmul writing both role
            # slots (rhs = 2-row strided AP, out = both bank slots).
            TILEOF = {"E0A": ("E0AB", 0), "E0B": ("E0AB", 2),
                      "E1": ("E1", 0), "E2": ("E2", 0)}
            BLOCK = {"E0AB": 0, "E1": 1, "E2": 2}
            BANK_ORDER = {0: 0, 1: 1, 2: 2, 4: 3, 3: 4}  # bank3 (r8) last
            from collections import defaultdict as _dd
            mm_descs = []  # (block, bank, s, tilekey, rows=[(slot, gt, c0), ..])
            for bank in range(5):
                slots = sorted(
                    (sl, r) for r, (b, sl) in ROLE_SLOT.items() if b == bank
                )
                per = []
                for sl, r in slots:
                    g = _dd(list)
                    for tname, s, t, c0 in terms[(r // 3, r % 3)]:
                        tkey, toff = TILEOF[tname]
                        g[(tkey, s)].append((sl, toff + t, c0))
                    per.append(g)
                keys = set().union(*(p.keys() for p in per))
                for tkey, s in sorted(keys):
                    lists = [p.get((tkey, s), []) for p in per]
                    a = lists[0]
                    b_ = lists[1] if len(lists) > 1 else []
                    for ra, rb in zip(a, b_):
                        mm_descs.append((BLOCK[tkey], bank, s, tkey, [ra, rb]))
                    for row in a[len(b_):] + b_[len(a):]:
                        mm_descs.append((BLOCK[tkey], bank, s, tkey, [row]))
            mm_descs.sort(key=lambda m: (m[0], int(m[0] == 2 and max(r[1] for r in m[4]) > 2), BANK_ORDER[m[1]], m[2]))
            bank_last = {}
            for idx, m in enumerate(mm_descs):
                bank_last[m[1]] = idx
            started = set()
            emitted = [0]

            def emit_roles(blockidx):
                for idx, (blk, bank, s, tkey, rows) in enumerate(mm_descs):
                    if blk != blockidx:
                        continue
                    Eg, k = E[tkey]
                    base = Eg[0:k, rows[0][1], rows[0][2] : rows[0][2] + WOUT]
                    if len(rows) == 2:
                        stride = (rows[1][1] - rows[0][1]) * W + (
                            rows[1][2] - rows[0][2]
                        )
                        rhs = _ap(base, [[stride, 2], [1, WOUT]])
                        out = _ap(S[bank][:, 0, 0:WOUT], [[WOUT, 2], [1, WOUT]])
                    else:
                        rhs = base
                        out = S[bank][:, rows[0][0], :]
                    nc.tensor.matmul(
                        out,
                        WS[0:k, s, :],
                        rhs,
                        start=(bank not in started),
                        stop=(idx == bank_last[bank]),
                        skip_group_check=True,
                    )
                    started.add(bank)
                    emitted[0] += 1

            def subs_pair(D, pbase, P, xa, xb, c):
                """planes (pbase, pbase+1) = same-row pairs dc=1,2 via a
                negative-stride 2-plane operand (plane1@PAD, plane0@PAD+2)."""
                anchor = xa[0:P, 0, c, PAD : PAD + W]
                nc.vector.tensor_sub(
                    _ap(D[c][:, pbase, 0:W], [[W, 2], [1, W]]),
                    _ap(anchor, [[0, 2], [1, W]]),
                    _ap(xb[0:P, 1, c, PAD : PAD + W], [[-(C * WT - 2), 2], [1, W]]),
                )

            def subs_wide(D, P, xa, xb, c):
                """five planes dc=-2..2 at a row gap (xb = shifted-row tile)."""
                a1 = xa[0:P, 0, c, PAD : PAD + W].unsqueeze(1)
                nc.vector.tensor_sub(
                    _ap(D[c][:, 0, 0:W], [[2 * W, 3], [1, W]]),
                    a1.to_broadcast([P, 3, W]),
                    _ap(xb[0:P, 0, c, PAD - 2 : PAD - 2 + W], [[2, 3], [1, W]]),
                )
                nc.vector.tensor_sub(
                    _ap(D[c][:, 1, 0:W], [[2 * W, 2], [1, W]]),
                    a1.to_broadcast([P, 2, W]),
                    _ap(xb[0:P, 1, c, PAD - 2 : PAD - 2 + W], [[2, 2], [1, W]]),
                )

            def flat(tile_, P, off, n):
                return _ap(tile_[0:P, off // W, 0 : min(n, W)], [[1, n]])

            def d2_pe_exp(name, D, Eg, P, h0, h1, q_eng):
                """squares, then per 512-chunk: 3 accumulating identity
                matmuls -> PSUM d2, Exp(PSUM) -> Eg slice."""
                hn = h1 - h0
                q = []
                for c in range(C):
                    qc = dp.tile([P, hn, W], BF16, tag=f"q{c}_{name}",
                                 name=f"q{c}_{name}")
                    if q_eng[c] == "v":
                        nc.vector.tensor_mul(qc, D[c][:, h0:h1, :], D[c][:, h0:h1, :])
                    else:
                        nc.scalar.square(qc, D[c][:, h0:h1, :])
                    q.append(qc)
                total = hn * W
                for a in range(0, total, 512):
                    n = min(512, total - a)
                    d2c = pp.tile([128, 512], F32, tag="d2c", bufs=2, name=f"d2_{name}_{a}")
                    for ci, qc in enumerate(q):
                        nc.tensor.matmul(
                            d2c[:, 0:n],
                            WS[0:P, 0, :],
                            flat(qc, P, a, n),
                            start=(ci == 0),
                            stop=(ci == C - 1),
                            skip_group_check=True,
                        )
                    nc.scalar.activation(
                        flat(Eg, P, h0 * W + a, n), d2c[0:P, 0:n], AF.Exp, scale=-2.0
                    )

            def d2_v_exp(name, D, Eg, P, nb, q_eng, exp_halves):
                """squares, d2 via two vector adds, Exp per half from SBUF."""
                q = []
                for c in range(C):
                    qc = dp.tile([P, nb, W], BF16, tag=f"q{c}_{name}",
                                 name=f"q{c}_{name}")
                    if q_eng[c] == "v":
                        nc.vector.tensor_mul(qc, D[c], D[c])
                    else:
                        nc.scalar.square(qc, D[c])
                    q.append(qc)
                d2a = dp.tile([P, nb, W], BF16, tag=f"d2a_{name}")
                nc.vector.tensor_add(d2a, q[0], q[1])
                d2 = dp.tile([P, nb, W], BF16, tag=f"d2_{name}")
                nc.vector.tensor_add(d2, d2a, q[2])
                for h0, h1 in exp_halves:
                    nc.scalar.activation(
                        Eg[:, h0:h1, :], d2[:, h0:h1, :], AF.Exp, scale=-2.0
                    )

            # E0AB: planes 0,1 = E0A (rows 0..127); planes 2,3 = E0B (rows 1..128)
            D0 = [dp.tile([128, 4, W], BF16, tag=f"d_E0AB_{c}", name=f"d_E0AB_{c}")
                  for c in range(C)]
            for c in range(C):
                subs_pair(D0, 0, 128, XX[0], XX[0], c)
                subs_pair(D0, 2, 128, XX[1], XX[1], c)
            E0AB = ep.tile([128, 4, W], BF16, tag="e_E0AB")
            d2_pe_exp("E0AB", D0, E0AB, 128, 0, 4, "vss")
            E["E0AB"] = (E0AB, 128)
            emit_roles(0)

            D1 = [dp.tile([128, 5, W], BF16, tag=f"d_E1_{c}", name=f"d_E1_{c}")
                  for c in range(C)]
            for c in range(C):
                subs_wide(D1, 128, XX[0], XX[1], c)
            E1T = ep.tile([128, 5, W], BF16, tag="e_E1")
            d2_v_exp("E1", D1, E1T, 128, 5, "vvs", ((0, 5),))
            E["E1"] = (E1T, 128)
            emit_roles(1)

            D2 = [dp.tile([127, 5, W], BF16, tag=f"d_E2_{c}", name=f"d_E2_{c}")
                  for c in range(C)]
            for c in range(C):
                subs_wide(D2, 127, XX[0], XX[2], c)
            E2T = ep.tile([127, 5, W], BF16, tag="e_E2")
            d2_v_exp("E2", D2, E2T, 127, 5, "vvv", ((0, 3), (3, 5)))
            E["E2"] = (E2T, 127)
            emit_roles(2)

            # ---- stage D: ln per role (bank-stop order), 9-plane sum as
            # accumulating identity matmuls into recycled bank 0, one scalar
            # copy, DMA. Host applies out = -sum/9 + LOG_NORM. -------------
            LT = sp.tile([128, 9, WOUT], BF16, tag="lt")
            ln_order = [3, 4, 1, 2, 6, 7, 0, 5, 8]
            for r in ln_order:
                bank, slot = ROLE_SLOT[r]
                nc.scalar.activation(LT[:, r, :], S[bank][:, slot, :], AF.Ln, bias=1.0)
            for i, r in enumerate(ln_order):
                nc.tensor.matmul(
                    S[0][:, 0, :],
                    WS[:, 0, :],
                    LT[:, r, :],
                    start=(i == 0),
                    stop=(i == 8),
                    skip_group_check=True,
                )
            OUTT = sp.tile([128, WOUT], BF16, tag="out")
            nc.scalar.copy(OUTT, S[0][:, 0, :])
            nc.sync.dma_start(out=yout[:, :], in_=OUTT)
    if not nc.is_finalized():
        with _one_act_table():
            nc.finalize()
    return nc


_PROGRAM = None


def _get_program():
    global _PROGRAM
    if _PROGRAM is None:
        _PROGRAM = _build_program()
    return _PROGRAM


def _make_shift_weights():
    w = np.zeros((128, 2, 128), dtype=ml_dtypes.float8_e4m3)
    for s in range(2):
        for m in range(128):
            if m + s < 128:
                w[m + s, s, m] = 1.0
    return w


def _shard_inputs(x):
    x = np.asarray(x, dtype=np.float32)
    # [B, rows(257: 256 + pad row), 2(plain, col-shifted), C, WT]
    xp = np.zeros((B, 257, 2, C, WT), dtype=np.float32)
    xp[:, :256, 0, :, PAD : PAD + W] = x.transpose(0, 2, 1, 3)
    xp[:, :, 1, :, : WT - 1] = xp[:, :, 0, :, 1:]
    xp16 = xp.astype(ml_dtypes.bfloat16)
    wsh = _make_shift_weights()
    in_maps = []
    for core in range(8):
        b, half = divmod(core, 2)
        r0 = half * 127
        in_maps.append(
            {
                "xin": np.ascontiguousarray(xp16[b, r0 : r0 + ROWS_IN]),
                "wsh": wsh,
            }
        )
    return in_maps


def _gather(results):
    out = np.empty((B, 254, 254), dtype=np.float32)
    for core in range(8):
        b, half = divmod(core, 2)
        lt = np.asarray(results[core]["yout"][:127], dtype=np.float32)
        out[b, half * 127 : half * 127 + 127, :] = lt * (-1.0 / 9.0) + LOG_NORM
    return out


def kernel(x, **_unused):
    nc = _get_program()
    res = run_bass_kernel_spmd(nc, _shard_inputs(x), core_ids=list(range(8)))
    return _gather(res.results)


def kernel_traced(x):
    """Same as kernel() but returns (output, BassKernelResults) with trace."""
    nc = _get_program()
    res = run_bass_kernel_spmd(
        nc, _shard_inputs(x), core_ids=list(range(8)), trace=True
    )
    return _gather(res.results), res
